# revision 20
# baseline (speedup 1.0000x reference)
"""DGCNN-style EdgeConv point-cloud network on 8 Trainium2 NeuronCores.

Math trick: edge = [center, neigh-center] @ W decomposes as
    h[n,k] = center[n] @ (Wt - Wb) + neigh[n,k] @ Wb        (Wt = W[:C], Wb = W[C:])
so per-layer work collapses to two point-level matmuls (A = F@(Wt-Wb), Bm = F@Wb)
plus a gather of Bm rows by kNN index and a max over the 16 neighbors:
    h_max[n] = A[n] + max_k Bm[idx[n,k]].
Biases fold into the (training-mode) BN shift; BN stats are all-reduced across
the 8 cores (data-parallel over batch; each cloud is processed by two cores,
which leaves the mean/var unchanged).

Sharding: core c processes cloud c % 4 fully. Host gathers outputs of cores 0-3.
"""

import numpy as np

import concourse.bass as bass
import concourse.masks as masks
import concourse.tile as tile
from concourse import bacc, mybir
from concourse.bass_utils import run_bass_kernel_spmd

F32 = mybir.dt.float32
U16 = mybir.dt.uint16
I16 = mybir.dt.int16

B, N, D, KNN = 4, 1024, 3, 16
FEATURE_DIMS = [64, 128, 256]
EMB = 512
NCORES = 8
NEG = -1.0e30
EPS = 1e-5
SLOPE = 0.2
NPTS = NCORES * N          # BN denominator: 8 cores x 1024 pts (each cloud twice)
NT = N // 128              # 8 row tiles of 128 points
GCHUNKS = 8                # gather chunks per layer
IDX_PER_CHUNK = N * KNN // GCHUNKS
DEBUG = False
REPEAT = 1
USE_CC = True
USE_GATHER = True


def _canon_out(ap2d, q):
    """Strided out-view placing natural tile-q columns (m = 0..127, point
    n = 128q + m) at canonical positions c(n) = ((n%128)//16)*128 + 16*(n//128)
    + n%16 = (m//16)*128 + 16*q + (m%16): dims [(128,8) m//16, (1,16) m%16]."""
    return bass.AP(tensor=ap2d.tensor, offset=ap2d.offset + 16 * q,
                   ap=[list(ap2d.ap[0]), [128, 8], [1, 16]])


def _sigma_out(ap2d):
    """Strided out-view writing natural column m of tile r to position
    r*128 + sigma^-1(m), sigma^-1(m) = 8*(m%16) + m//16, so that psum row p
    of the distance matmul holds point n = r*128 + sigma(p),
    sigma(p) = 16*(p%8) + p//8."""
    return bass.AP(tensor=ap2d.tensor, offset=ap2d.offset,
                   ap=[list(ap2d.ap[0]), [128, 8], [1, 8], [8, 16]])


def _bcast_q(ap2d, q):
    """View a [128, C] AP as [128, q, C] with a step-0 middle dim."""
    return bass.AP(tensor=ap2d.tensor, offset=ap2d.offset,
                   ap=[list(ap2d.ap[0]), [0, q], list(ap2d.ap[1])])


def build_program(nc, tc, tensors, ctx):
    xyz = tensors["xyz"]
    out_t = tensors["out"]

    cpool = ctx.enter_context(tc.tile_pool(name="const", bufs=1))
    tpool = ctx.enter_context(tc.tile_pool(name="topk", bufs=2))
    apool = ctx.enter_context(tc.tile_pool(name="acts", bufs=1))
    spool = ctx.enter_context(tc.tile_pool(name="smax", bufs=1))
    bpool = ctx.enter_context(tc.tile_pool(name="bm", bufs=1))
    qpool = ctx.enter_context(tc.tile_pool(name="sq", bufs=2))
    gpool = ctx.enter_context(tc.tile_pool(name="gath", bufs=2))
    fpool = ctx.enter_context(tc.tile_pool(name="ft", bufs=4))
    mpool = ctx.enter_context(tc.tile_pool(name="misc", bufs=1))
    dpool = ctx.enter_context(tc.tile_pool(name="dram", bufs=2, space="DRAM"))
    bmdram = ctx.enter_context(tc.tile_pool(name="bmdram", bufs=1, space="DRAM"))

    pdist = ctx.enter_context(tc.tile_pool(name="pdist", bufs=1, space="PSUM"))
    pab = ctx.enter_context(tc.tile_pool(name="pab", bufs=2, space="PSUM"))
    pT = ctx.enter_context(tc.tile_pool(name="pT", bufs=1, space="PSUM"))
    pstat = ctx.enter_context(tc.tile_pool(name="pstat", bufs=2, space="PSUM"))

    # ---------------- constants / weights ----------------
    ident = cpool.tile([128, 128], F32)
    masks.make_identity(nc, ident[:])
    ones_col = cpool.tile([128, 1], F32)
    nc.gpsimd.memset(ones_col[:], 1.0)
    ones_row = cpool.tile([1, 128], F32)
    nc.gpsimd.memset(ones_row[:], 1.0)

    # per-layer weight tiles: Wt/Wb K-chunks loaded separately from DRAM
    wsb = {}
    for li, (cin, cout) in enumerate(zip([3] + FEATURE_DIMS[:-1], FEATURE_DIMS), start=1):
        w = tensors[f"W{li}"]
        nch = (cin + 127) // 128
        wt_tiles, wb_tiles = [], []
        for kc in range(nch):
            rows = min(128, cin - kc * 128)
            wt = cpool.tile([rows, cout], F32, tag=f"Wt{li}_{kc}")
            wb = cpool.tile([rows, cout], F32, tag=f"Wb{li}_{kc}")
            nc.sync.dma_start(wt[:], w.ap()[kc * 128:kc * 128 + rows, :])
            nc.sync.dma_start(wb[:], w.ap()[cin + kc * 128:cin + kc * 128 + rows, :])
            wt_tiles.append(wt)
            wb_tiles.append(wb)
        wsb[li] = (wt_tiles, wb_tiles)
    we_tiles = []
    for j in range(2):
        t = cpool.tile([128, EMB], F32, tag=f"We_{j}")
        nc.sync.dma_start(t[:], tensors["We"].ap()[j * 128:(j + 1) * 128, :])
        we_tiles.append(t)

    vec_sb = {}
    for name in ["b1", "g1", "be1", "b2", "g2", "be2", "b3", "g3", "be3",
                 "bse", "ge", "bee"]:
        c = tensors[name].shape[1]
        t = cpool.tile([1, c], F32, tag=f"vec_{name}")
        nc.sync.dma_start(t[:], tensors[name].ap())
        vec_sb[name] = t

    # Wd = Wt - Wb chunks ([C_in_chunk, C_out] each)
    def make_wd(li, cin, cout):
        wt_tiles, wb_tiles = wsb[li]
        chunks = []
        for kc, (wt, wb) in enumerate(zip(wt_tiles, wb_tiles)):
            rows = wt.shape[0]
            wd = cpool.tile([rows, cout], F32, tag=f"wd{li}_{kc}")
            nc.vector.tensor_sub(wd[:], wt[:], wb[:])
            chunks.append((wd[:], wb[:]))  # (Wd, Wb)
        return chunks

    wd_chunks = {1: make_wd(1, 3, 64), 2: make_wd(2, 64, 128), 3: make_wd(3, 128, 256)}

    # ---------------- xyz load + transpose ----------------
    xyz_nat = cpool.tile([128, 8, 3], F32)
    nc.sync.dma_start(xyz_nat[:], xyz.ap().rearrange("(q p) d -> p q d", p=128))
    XT = cpool.tile([3, N], F32)
    XTc = cpool.tile([3, N], F32)
    for q in range(NT):
        ps = pT.tile([3, 128], F32)
        nc.tensor.transpose(ps[:], xyz_nat[:, q, :], ident[:])
        nc.scalar.copy(XT[:][:, q * 128:(q + 1) * 128], ps[:])
        nc.scalar.copy(_canon_out(XTc[:], q), ps[:])

    # squared norms; build matmul operands for negD = 2<xn,xm> - |xm|^2
    xtsq = cpool.tile([3, N], F32)
    nc.scalar.square(xtsq[:], XT[:])
    neg_ones3 = cpool.tile([3, 1], F32)
    nc.gpsimd.memset(neg_ones3[:], -1.0)
    rhs4 = cpool.tile([4, N], F32)
    nc.vector.tensor_copy(rhs4[:][0:3, :], XT[:])
    negsq1 = cpool.tile([1, N], F32)
    for half in range(2):
        psn = pstat.tile([1, 512], F32, tag="pstat")
        nc.tensor.matmul(psn[:], neg_ones3[:], xtsq[:][:, half * 512:(half + 1) * 512])
        nc.scalar.copy(negsq1[:][:, half * 512:(half + 1) * 512], psn[:])
    nc.sync.dma_start(rhs4[:][3:4, :], negsq1[:])
    lhsT4 = cpool.tile([4, N], F32)
    nc.scalar.mul(_sigma_out(lhsT4[:][0:3, :]), XT[:], 2.0)
    ones1 = cpool.tile([1, N], F32)
    nc.gpsimd.memset(ones1[:], 1.0)
    nc.sync.dma_start(lhsT4[:][3:4, :], ones1[:])

    # ---------------- top-16 neighbors ----------------
    idx_all = cpool.tile([128, 128], U16)
    for r in range(NT):
        psD = pdist.tile([128, N], F32)
        lhs_ap = lhsT4[:][:, r * 128:(r + 1) * 128]
        nc.tensor.matmul(psD[:, 0:512], lhs_ap, rhs4[:][:, 0:512])
        nc.tensor.matmul(psD[:, 512:1024], lhs_ap, rhs4[:][:, 512:1024])
        negD = tpool.tile([128, N], F32, tag="negD")
        nc.scalar.copy(negD[:], psD[:])
        vals = tpool.tile([128, 16], F32, tag="vals")
        nc.vector.max(vals[:, 0:8], negD[:])
        nc.vector.max_index(idx_all[:][:, r:r + 57:8], vals[:, 0:8], negD[:])
        negD2 = tpool.tile([128, N], F32, tag="negD2")
        nc.vector.match_replace(negD2[:], vals[:, 0:8], negD[:], NEG)
        nc.vector.max(vals[:, 8:16], negD2[:])
        nc.vector.max_index(idx_all[:][:, 64 + r:64 + r + 57:8], vals[:, 8:16], negD2[:])

    # wrapped index layout for dma_gather: [16 partitions, 1024] u16, replicated x8
    wrapped = cpool.tile([128, N * KNN // 16], U16)
    for k in range(8):
        nc.sync.dma_start(wrapped[:][16 * k:16 * (k + 1), :], idx_all[:])
    if DEBUG:
        d_idx = nc.dram_tensor("dbg_idx", [128, 128], U16, kind="ExternalOutput")
        nc.sync.dma_start(d_idx.ap(), idx_all[:])
        d_wr = nc.dram_tensor("dbg_wrapped", [128, N * KNN // 16], U16, kind="ExternalOutput")
        nc.sync.dma_start(d_wr.ap(), wrapped[:])
        d_xt = nc.dram_tensor("dbg_XT", [3, N], F32, kind="ExternalOutput")
        nc.sync.dma_start(d_xt.ap(), XT[:])
        d_xtc = nc.dram_tensor("dbg_XTc", [3, N], F32, kind="ExternalOutput")
        nc.sync.dma_start(d_xtc.ap(), XTc[:])
        d_l4 = nc.dram_tensor("dbg_lhsT4", [4, N], F32, kind="ExternalOutput")
        nc.sync.dma_start(d_l4.ap(), lhsT4[:])
        d_r4 = nc.dram_tensor("dbg_rhs4", [4, N], F32, kind="ExternalOutput")
        nc.sync.dma_start(d_r4.ap(), rhs4[:])

    # ---------------- generic layer ----------------
    def emit_layer(ft_chunks, cin, cout, li):
        """ft_chunks: list of [K<=128, 1024] f32 APs (transposed features).
        Returns list of ft APs for the next layer."""
        chunks = wd_chunks[li]

        # A and Bm, both over canonical point tiles (ft is stored canonical)
        A = apool.tile([128, 8, cout], F32, tag="A")
        Bm = bpool.tile([128, 8, cout], F32, tag="Bm")
        for g in range(8):
            gs = slice(g * 128, (g + 1) * 128)
            psA = pab.tile([128, cout], F32, tag="psab")
            for kc, (wd, _) in enumerate(chunks):
                nc.tensor.matmul(psA[:], ft_chunks[kc][:, gs], wd,
                                 start=(kc == 0), stop=(kc == len(chunks) - 1))
            nc.scalar.copy(A[:, g, :], psA[:])
            psB = pab.tile([128, cout], F32, tag="psab")
            for kc, (_, wb) in enumerate(chunks):
                nc.tensor.matmul(psB[:], ft_chunks[kc][:, gs], wb,
                                 start=(kc == 0), stop=(kc == len(chunks) - 1))
            nc.scalar.copy(Bm[:, g, :], psB[:])

        # scatter canonical tiles to natural DRAM rows n = 128*(P//16) + 16g + P%16
        bm_d = bmdram.tile([N, cout], F32, tag="bmd")
        for g in range(8):
            dst = bass.AP(tensor=bm_d.tensor, offset=bm_d.offset + 16 * g * cout,
                          ap=[[128 * cout, 8], [cout, 16], [1, cout]])
            nc.sync.dma_start(dst, Bm[:, g, :])

        # gather + max over 16 neighbors
        S = spool.tile([128, 8, cout], F32, tag="S")
        wslice = N * KNN // 16 // GCHUNKS
        for cc in range(GCHUNKS):
            gt = gpool.tile([128, IDX_PER_CHUNK // 128, cout], F32, tag="gath")
            if USE_GATHER:
                nc.gpsimd.dma_gather(
                    gt[:], bm_d[:],
                    wrapped[:][:, cc * wslice:(cc + 1) * wslice].bitcast(I16),
                    num_idxs=IDX_PER_CHUNK, num_idxs_reg=IDX_PER_CHUNK,
                    elem_size=cout, single_packet=False)
            else:
                for jj in range(IDX_PER_CHUNK // 128 // 8):
                    nc.sync.dma_start(gt[:, jj * 8:(jj + 1) * 8, :], Bm[:])
            nc.vector.tensor_reduce(
                S[:, cc, :],
                gt[:].rearrange("p (gl t) c -> p gl c t", t=16),
                axis=mybir.AxisListType.X, op=mybir.AluOpType.max)

        if DEBUG and li == 1:
            d_a = nc.dram_tensor("dbg_A1", [128, 8 * cout], F32, kind="ExternalOutput")
            nc.sync.dma_start(d_a.ap(), A[:])
            d_s = nc.dram_tensor("dbg_S1", [128, 8 * cout], F32, kind="ExternalOutput")
            nc.sync.dma_start(d_s.ap(), S[:])
            d_b = nc.dram_tensor("dbg_Bmd1", [N, cout], F32, kind="ExternalOutput")
            nc.sync.dma_start(d_b.ap(), bm_d[:])
        # h_pre = A + S (in place into A)
        nc.vector.tensor_add(A[:], A[:], S[:])

        # stats: sum and sum of squares over points (PE ones-trick)
        ps1 = pstat.tile([1, cout], F32, tag="pstat")
        ps2 = pstat.tile([1, cout], F32, tag="pstat")
        for g in range(8):
            nc.tensor.matmul(ps1[:], ones_col[:], A[:, g, :],
                             start=(g == 0), stop=(g == 7))
        for g in range(8):
            sg = qpool.tile([128, cout], F32, tag="sqg")
            nc.scalar.square(sg[:], A[:, g, :])
            nc.tensor.matmul(ps2[:], ones_col[:], sg[:],
                             start=(g == 0), stop=(g == 7))
        return finish_bn(A, ps1, ps2, cout, li=li)

    def finish_bn(hp, ps1, ps2, cout, li):
        """AllReduce stats, compute scale/shift, apply BN + leaky relu to hp
        ([128, 8, cout]), transpose to ft chunks. li=0 means embedding layer."""
        bname, gname, bename = (f"b{li}", f"g{li}", f"be{li}") if li else ("bse", "ge", "bee")
        stat = mpool.tile([1, 2 * cout], F32, tag="stat")
        nc.scalar.copy(stat[:, 0:cout], ps1[:])
        nc.scalar.copy(stat[:, cout:2 * cout], ps2[:])
        cin_d = dpool.tile([1, 2 * cout], F32, tag="cc_in")
        cout_d = dpool.tile([1, 2 * cout], F32, tag="cc_out")
        nc.gpsimd.dma_start(cin_d[:], stat[:])
        if USE_CC:
            nc.gpsimd.collective_compute(
                "AllReduce", mybir.AluOpType.add,
                replica_groups=[list(range(NCORES))],
                ins=[cin_d.opt()], outs=[cout_d.opt()])
        else:
            nc.gpsimd.dma_start(cout_d[:], cin_d[:])
        statg = mpool.tile([1, 2 * cout], F32, tag="statg")
        nc.gpsimd.dma_start(statg[:], cout_d[:])

        # scale/shift on one partition
        ss = mpool.tile([1, 2 * cout], F32, tag="ss")
        mean = mpool.tile([1, cout], F32, tag="mean")
        var = mpool.tile([1, cout], F32, tag="var")
        nc.scalar.mul(mean[:], statg[:, 0:cout], 1.0 / NPTS)
        nc.scalar.mul(var[:], statg[:, cout:2 * cout], 1.0 / NPTS)   # E[x^2]
        msq = mpool.tile([1, cout], F32, tag="msq")
        nc.scalar.square(msq[:], mean[:])
        # var = (E[x^2] + eps) - mean^2   then   sqrt
        nc.vector.scalar_tensor_tensor(var[:], var[:], EPS, msq[:],
                                       op0=mybir.AluOpType.add,
                                       op1=mybir.AluOpType.subtract)
        nc.scalar.activation(msq[:], var[:], mybir.ActivationFunctionType.Sqrt)
        nc.vector.reciprocal(var[:], msq[:])                         # 1/sqrt(var+eps)
        scale_ap, shift_ap = ss[:, 0:cout], ss[:, cout:2 * cout]
        nc.vector.tensor_mul(scale_ap, var[:], vec_sb[gname][:])
        nc.vector.tensor_add(msq[:], mean[:], vec_sb[bname][:])      # mean + b
        nc.vector.tensor_mul(msq[:], msq[:], scale_ap)
        nc.vector.tensor_sub(shift_ap, vec_sb[bename][:], msq[:])

        # broadcast scale/shift to 128 partitions via PE
        ssr = mpool.tile([128, 2 * cout], F32, tag="ssr")
        for half in range(2):
            psb = pab.tile([128, cout], F32, tag="psab")
            nc.tensor.matmul(psb[:], ones_row[:], ss[:, half * cout:(half + 1) * cout])
            nc.scalar.copy(ssr[:, half * cout:(half + 1) * cout], psb[:])

        # apply: h = lrelu(hp * scale + shift)
        nc.vector.tensor_mul(hp[:], hp[:], _bcast_q(ssr[:, 0:cout], 8))
        nc.vector.tensor_add(hp[:], hp[:], _bcast_q(ssr[:, cout:2 * cout], 8))
        nc.vector.scalar_tensor_tensor(hp[:], hp[:], SLOPE, hp[:],
                                       op0=mybir.AluOpType.mult,
                                       op1=mybir.AluOpType.max)
        if DEBUG and li == 1:
            d_h = nc.dram_tensor("dbg_h1", [128, 8 * hp.shape[2]], F32, kind="ExternalOutput")
            nc.sync.dma_start(d_h.ap(), hp[:])
        if li == 0:
            return hp

        # transpose h -> ft chunks for next layer
        nch = (cout + 127) // 128
        fts = []
        for oc in range(nch):
            cw = min(128, cout - oc * 128)
            ft = fpool.tile([cw, N], F32, tag="ft")
            for g in range(8):
                pst = pT.tile([128, 128], F32, tag="pT")
                nc.tensor.transpose(pst[:cw, :], hp[:, g, oc * 128:oc * 128 + cw], ident[:])
                nc.scalar.copy(ft[:][:, g * 128:(g + 1) * 128], pst[:cw, :])
            fts.append(ft[:])
        return fts

    ft = [XTc[:]]
    ft = emit_layer(ft, 3, 64, 1)
    ft = emit_layer(ft, 64, 128, 2)
    ft = emit_layer(ft, 128, 256, 3)

    # ---------------- embedding + global max pool ----------------
    e = qpool.tile([128, 8, EMB], F32, tag="e")
    for g in range(8):
        pse = pab.tile([128, EMB], F32, tag="psab")
        for kc in range(2):
            nc.tensor.matmul(pse[:], ft[kc][:, g * 128:(g + 1) * 128], we_tiles[kc][:],
                             start=(kc == 0), stop=(kc == 1))
        nc.scalar.copy(e[:, g, :], pse[:])
    ps1 = pstat.tile([1, EMB], F32, tag="pstat")
    ps2 = pstat.tile([1, EMB], F32, tag="pstat")
    for g in range(8):
        nc.tensor.matmul(ps1[:], ones_col[:], e[:, g, :], start=(g == 0), stop=(g == 7))
    for g in range(8):
        sg = qpool.tile([128, EMB], F32, tag="sqg")
        nc.scalar.square(sg[:], e[:, g, :])
        nc.tensor.matmul(ps2[:], ones_col[:], sg[:], start=(g == 0), stop=(g == 7))
    e = finish_bn(e, ps1, ps2, EMB, li=0)

    # max over the 8 groups, then over 128 partitions (PE transpose + DVE reduce)
    m1 = mpool.tile([128, EMB], F32, tag="m1")
    nc.vector.tensor_reduce(m1[:], e[:].rearrange("p g c -> p c g"),
                            axis=mybir.AxisListType.X, op=mybir.AluOpType.max)
    m2 = mpool.tile([128, 4], F32, tag="m2")
    for j in range(4):
        pst = pT.tile([128, 128], F32, tag="pT")
        nc.tensor.transpose(pst[:], m1[:, j * 128:(j + 1) * 128], ident[:])
        ts = mpool.tile([128, 128], F32, tag="ts")
        nc.scalar.copy(ts[:], pst[:])
        nc.vector.tensor_reduce(m2[:, j:j + 1], ts[:],
                                axis=mybir.AxisListType.X, op=mybir.AluOpType.max)
    psf = pT.tile([4, 128], F32, tag="pT")
    nc.tensor.transpose(psf[:], m2[:], ident[:])
    fin = mpool.tile([4, 128], F32, tag="fin")
    nc.scalar.copy(fin[:], psf[:])
    nc.sync.dma_start(out_t.ap().rearrange("o (j c) -> o j c", j=4), fin[:])


_CACHE = {}


def _build():
    if "nc" in _CACHE:
        return _CACHE["nc"]
    nc = bacc.Bacc("TRN2", target_bir_lowering=False, debug=False,
                   enable_asserts=False, num_devices=NCORES)
    tensors = {"xyz": nc.dram_tensor("xyz", [N, D], F32, kind="ExternalInput"),
               "out": nc.dram_tensor("out", [1, EMB], F32, kind="ExternalOutput")}
    cin = D
    for li, cdim in enumerate(FEATURE_DIMS, start=1):
        tensors[f"W{li}"] = nc.dram_tensor(f"W{li}", [2 * cin, cdim], F32, kind="ExternalInput")
        for pre in ("b", "g", "be"):
            tensors[f"{pre}{li}"] = nc.dram_tensor(f"{pre}{li}", [1, cdim], F32, kind="ExternalInput")
        cin = cdim
    tensors["We"] = nc.dram_tensor("We", [cin, EMB], F32, kind="ExternalInput")
    for nm in ("bse", "ge", "bee"):
        tensors[nm] = nc.dram_tensor(nm, [1, EMB], F32, kind="ExternalInput")

    from contextlib import ExitStack
    with tile.TileContext(nc) as tc:
        for _rep in range(REPEAT):
            with ExitStack() as ctx:
                build_program(nc, tc, tensors, ctx)
    nc.compile()
    _CACHE["nc"] = nc
    return nc


def _in_maps(inputs):
    shared = {}
    for k, v in inputs.items():
        if k == "xyz":
            continue
        a = np.ascontiguousarray(np.asarray(v, dtype=np.float32))
        if a.ndim == 1:
            a = a[None, :]
        shared[k] = a
    xyz = np.asarray(inputs["xyz"], dtype=np.float32)
    maps = []
    for c in range(NCORES):
        m = dict(shared)
        m["xyz"] = np.ascontiguousarray(xyz[c % B])
        maps.append(m)
    return maps


def kernel(**inputs):
    nc = _build()
    res = run_bass_kernel_spmd(nc, _in_maps(inputs), core_ids=list(range(NCORES)))
    return np.stack([res.results[c]["out"][0] for c in range(B)])


# revision 28
# speedup vs baseline: 1.6517x; 1.6517x over previous
"""DGCNN-style EdgeConv point-cloud network on 8 Trainium2 NeuronCores.

Math trick: edge = [center, neigh-center] @ W decomposes as
    h[n,k] = center[n] @ (Wt - Wb) + neigh[n,k] @ Wb        (Wt = W[:C], Wb = W[C:])
so per-layer work collapses to two point-level matmuls (A = F@(Wt-Wb), Bm = F@Wb)
plus a gather of Bm rows by kNN index and a max over the 16 neighbors:
    h_max[n] = A[n] + max_k Bm[idx[n,k]].
Biases fold into the (training-mode) BN shift; BN stats are all-reduced across
the 8 cores (data-parallel over batch; each cloud is processed by two cores,
which leaves the mean/var unchanged).

Sharding: core c processes cloud c % 4 fully. Host gathers outputs of cores 0-3.
"""

import numpy as np

import concourse.bass as bass
import concourse.masks as masks
import concourse.tile as tile
from concourse import bacc, mybir
from concourse.bass_utils import run_bass_kernel_spmd

F32 = mybir.dt.float32
BF16 = mybir.dt.bfloat16
U16 = mybir.dt.uint16
I16 = mybir.dt.int16

B, N, D, KNN = 4, 1024, 3, 16
FEATURE_DIMS = [64, 128, 256]
EMB = 512
NCORES = 8
NEG = -1.0e30
EPS = 1e-5
SLOPE = 0.2
NPTS = NCORES * N          # BN denominator: 8 cores x 1024 pts (each cloud twice)
NT = N // 128              # 8 row tiles of 128 points
GCHUNKS = 8                # gather chunks per layer
IDX_PER_CHUNK = N * KNN // GCHUNKS
DEBUG = False
REPEAT = 1
LRELU_ACT = False
USE_CC = True
USE_GATHER = True


def _canon_out(ap2d, q):
    """Strided out-view placing natural tile-q columns (m = 0..127, point
    n = 128q + m) at canonical positions c(n) = ((n%128)//16)*128 + 16*(n//128)
    + n%16 = (m//16)*128 + 16*q + (m%16): dims [(128,8) m//16, (1,16) m%16]."""
    return bass.AP(tensor=ap2d.tensor, offset=ap2d.offset + 16 * q,
                   ap=[list(ap2d.ap[0]), [128, 8], [1, 16]])


def _sigma_out(ap2d):
    """Strided out-view writing natural column m of tile r to position
    r*128 + sigma^-1(m), sigma^-1(m) = 8*(m%16) + m//16, so that psum row p
    of the distance matmul holds point n = r*128 + sigma(p),
    sigma(p) = 16*(p%8) + p//8."""
    return bass.AP(tensor=ap2d.tensor, offset=ap2d.offset,
                   ap=[list(ap2d.ap[0]), [128, 8], [1, 8], [8, 16]])


def _bcast_free(ap_col, n):
    """View a [P, 1] AP as [P, n] with a step-0 free dim."""
    return bass.AP(tensor=ap_col.tensor, offset=ap_col.offset,
                   ap=[list(ap_col.ap[0]), [0, n]])


def build_program(nc, tc, tensors, ctx):
    xyz = tensors["xyz"]
    out_t = tensors["out"]

    cpool = ctx.enter_context(tc.tile_pool(name="const", bufs=1))
    tpool = ctx.enter_context(tc.tile_pool(name="topk", bufs=2))
    apool = ctx.enter_context(tc.tile_pool(name="acts", bufs=1))
    spool = ctx.enter_context(tc.tile_pool(name="smax", bufs=1))
    bpool = ctx.enter_context(tc.tile_pool(name="bm", bufs=1))
    qpool = ctx.enter_context(tc.tile_pool(name="sq", bufs=2))
    gpool = ctx.enter_context(tc.tile_pool(name="gath", bufs=2))
    fpool = ctx.enter_context(tc.tile_pool(name="ft", bufs=6))
    mpool = ctx.enter_context(tc.tile_pool(name="misc", bufs=1))
    dpool = ctx.enter_context(tc.tile_pool(name="dram", bufs=2, space="DRAM"))
    bmdram = ctx.enter_context(tc.tile_pool(name="bmdram", bufs=1, space="DRAM"))

    pab = ctx.enter_context(tc.tile_pool(name="pab", bufs=2, space="PSUM"))
    pT = ctx.enter_context(tc.tile_pool(name="pT", bufs=1, space="PSUM"))
    pstat = ctx.enter_context(tc.tile_pool(name="pstat", bufs=2, space="PSUM"))

    # ---------------- constants / weights ----------------
    ident = cpool.tile([128, 128], F32)
    masks.make_identity(nc, ident[:])
    ones_col = cpool.tile([128, 1], F32)
    nc.gpsimd.memset(ones_col[:], 1.0)
    ones_row = cpool.tile([1, 128], F32)
    nc.gpsimd.memset(ones_row[:], 1.0)

    # per-layer weight tiles: Wt/Wb K-chunks loaded separately from DRAM
    wsb = {}
    for li, (cin, cout) in enumerate(zip([3] + FEATURE_DIMS[:-1], FEATURE_DIMS), start=1):
        w = tensors[f"W{li}"]
        nch = (cin + 127) // 128
        wt_tiles, wb_tiles = [], []
        for kc in range(nch):
            rows = min(128, cin - kc * 128)
            wt = cpool.tile([rows, cout], F32, tag=f"Wt{li}_{kc}")
            wb = cpool.tile([rows, cout], F32, tag=f"Wb{li}_{kc}")
            nc.sync.dma_start(wt[:], w.ap()[kc * 128:kc * 128 + rows, :])
            nc.sync.dma_start(wb[:], w.ap()[cin + kc * 128:cin + kc * 128 + rows, :])
            wt_tiles.append(wt)
            wb_tiles.append(wb)
        wsb[li] = (wt_tiles, wb_tiles)
    we_tiles = []
    for j in range(2):
        tf = cpool.tile([128, EMB], F32, tag=f"We_{j}")
        nc.sync.dma_start(tf[:], tensors["We"].ap()[j * 128:(j + 1) * 128, :])
        we_tiles.append(tf)

    vec_sb = {}
    for name in ["b1", "g1", "be1", "b2", "g2", "be2", "b3", "g3", "be3",
                 "bse", "ge", "bee"]:
        c = tensors[name].shape[1]
        t = cpool.tile([1, c], F32, tag=f"vec_{name}")
        nc.sync.dma_start(t[:], tensors[name].ap())
        vec_sb[name] = t

    # Wd = Wt - Wb chunks ([C_in_chunk, C_out] each)
    def make_wd(li, cin, cout):
        wt_tiles, wb_tiles = wsb[li]
        chunks = []
        for kc, (wt, wb) in enumerate(zip(wt_tiles, wb_tiles)):
            rows = wt.shape[0]
            wd = cpool.tile([rows, cout], F32, tag=f"wd{li}_{kc}")
            nc.vector.tensor_sub(wd[:], wt[:], wb[:])
            chunks.append((wd[:], wb[:]))  # (Wd, Wb)
        return chunks

    wd_chunks = {1: make_wd(1, 3, 64), 2: make_wd(2, 64, 128), 3: make_wd(3, 128, 256)}

    # ---------------- xyz load + transpose ----------------
    xyz_nat = cpool.tile([128, 8, 3], F32)
    nc.sync.dma_start(xyz_nat[:], xyz.ap().rearrange("(q p) d -> p q d", p=128))
    XT = cpool.tile([3, N], F32)
    XTc = cpool.tile([3, N], F32)
    for q in range(NT):
        ps = pT.tile([3, 128], F32, tag="pT")
        nc.tensor.transpose(ps[:], xyz_nat[:, q, :], ident[:])
        nc.scalar.copy(XT[:][:, q * 128:(q + 1) * 128], ps[:])
        nc.scalar.copy(_canon_out(XTc[:], q), ps[:])

    # squared norms; build matmul operands for negD = 2<xn,xm> - |xm|^2
    xtsq = cpool.tile([3, N], F32)
    nc.scalar.square(xtsq[:], XT[:])
    neg_ones3 = cpool.tile([3, 1], F32)
    nc.gpsimd.memset(neg_ones3[:], -1.0)
    rhs4 = cpool.tile([4, N], F32)
    nc.vector.tensor_copy(rhs4[:][0:3, :], XT[:])
    negsq1 = cpool.tile([1, N], F32)
    for half in range(2):
        psn = pstat.tile([1, 512], F32, tag="pstat")
        nc.tensor.matmul(psn[:], neg_ones3[:], xtsq[:][:, half * 512:(half + 1) * 512])
        nc.scalar.copy(negsq1[:][:, half * 512:(half + 1) * 512], psn[:])
    nc.sync.dma_start(rhs4[:][3:4, :], negsq1[:])
    lhsT4 = cpool.tile([4, N], F32)
    nc.scalar.mul(_sigma_out(lhsT4[:][0:3, :]), XT[:], 2.0)
    ones1 = cpool.tile([1, N], F32)
    nc.gpsimd.memset(ones1[:], 1.0)
    nc.sync.dma_start(lhsT4[:][3:4, :], ones1[:])

    # ---------------- top-16 neighbors ----------------
    idx_all = cpool.tile([128, 128], U16)
    for r in range(NT):
        lhs_ap = lhsT4[:][:, r * 128:(r + 1) * 128]
        negD = tpool.tile([128, N], F32, tag="negD")
        for hh in range(2):
            psD = pab.tile([128, 512], F32, tag="psab")
            nc.tensor.matmul(psD[:], lhs_ap, rhs4[:][:, hh * 512:(hh + 1) * 512])
            nc.scalar.copy(negD[:, hh * 512:(hh + 1) * 512], psD[:])
        vals = tpool.tile([128, 16], F32, tag="vals")
        nc.vector.max(vals[:, 0:8], negD[:])
        nc.vector.max_index(idx_all[:][:, r:r + 57:8], vals[:, 0:8], negD[:])
        negD2 = tpool.tile([128, N], F32, tag="negD2")
        nc.vector.match_replace(negD2[:], vals[:, 0:8], negD[:], NEG)
        nc.vector.max(vals[:, 8:16], negD2[:])
        nc.vector.max_index(idx_all[:][:, 64 + r:64 + r + 57:8], vals[:, 8:16], negD2[:])

    # wrapped index layout for dma_gather: [16 partitions, 1024] u16, replicated x8
    wrapped = cpool.tile([128, N * KNN // 16], U16)
    for k in range(8):
        nc.sync.dma_start(wrapped[:][16 * k:16 * (k + 1), :], idx_all[:])
    if DEBUG:
        d_idx = nc.dram_tensor("dbg_idx", [128, 128], U16, kind="ExternalOutput")
        nc.sync.dma_start(d_idx.ap(), idx_all[:])
        d_wr = nc.dram_tensor("dbg_wrapped", [128, N * KNN // 16], U16, kind="ExternalOutput")
        nc.sync.dma_start(d_wr.ap(), wrapped[:])
        d_xt = nc.dram_tensor("dbg_XT", [3, N], F32, kind="ExternalOutput")
        nc.sync.dma_start(d_xt.ap(), XT[:])
        d_xtc = nc.dram_tensor("dbg_XTc", [3, N], F32, kind="ExternalOutput")
        nc.sync.dma_start(d_xtc.ap(), XTc[:])
        d_l4 = nc.dram_tensor("dbg_lhsT4", [4, N], F32, kind="ExternalOutput")
        nc.sync.dma_start(d_l4.ap(), lhsT4[:])
        d_r4 = nc.dram_tensor("dbg_rhs4", [4, N], F32, kind="ExternalOutput")
        nc.sync.dma_start(d_r4.ap(), rhs4[:])

    # ---------------- generic layer ----------------
    def emit_layer(ft_chunks, cin, cout, li):
        """ft_chunks: list of [K<=128, 1024] bf16 APs (canonical transposed
        features). Returns next-layer ft chunks (bf16)."""
        chunks = wd_chunks[li]
        bmdt = F32  # bf16 gather misbehaves on HW; keep payload fp32

        A = apool.tile([128, 8, cout], F32, tag="A")
        Bm = bpool.tile([128, 8, cout], bmdt, tag="Bm")
        for g in range(8):
            gs = slice(g * 128, (g + 1) * 128)
            psA = pab.tile([128, cout], F32, tag="psab")
            for kc, (wd, _) in enumerate(chunks):
                nc.tensor.matmul(psA[:], ft_chunks[kc][:, gs], wd[:],
                                 start=(kc == 0), stop=(kc == len(chunks) - 1))
            nc.scalar.copy(A[:, g, :], psA[:])
            psB = pab.tile([128, cout], F32, tag="psab")
            for kc, (_, wb) in enumerate(chunks):
                nc.tensor.matmul(psB[:], ft_chunks[kc][:, gs], wb[:],
                                 start=(kc == 0), stop=(kc == len(chunks) - 1))
            nc.scalar.copy(Bm[:, g, :], psB[:])

        # scatter canonical tiles to natural DRAM rows n = 128*(P//16) + 16g + P%16
        bm_d = bmdram.tile([N, cout], bmdt, tag="bmd")
        for g in range(8):
            dst = bass.AP(tensor=bm_d.tensor, offset=bm_d.offset + 16 * g * cout,
                          ap=[[128 * cout, 8], [cout, 16], [1, cout]])
            nc.sync.dma_start(dst, Bm[:, g, :])

        # gather + max over 16 neighbors
        S = spool.tile([128, 8, cout], F32, tag="S")
        wslice = N * KNN // 16 // GCHUNKS
        for cc in range(GCHUNKS):
            gt = gpool.tile([128, IDX_PER_CHUNK // 128, cout], bmdt, tag="gath")
            nc.gpsimd.dma_gather(
                gt[:], bm_d[:],
                wrapped[:][:, cc * wslice:(cc + 1) * wslice].bitcast(I16),
                num_idxs=IDX_PER_CHUNK, num_idxs_reg=IDX_PER_CHUNK,
                elem_size=cout, single_packet=False)
            nc.vector.tensor_reduce(
                S[:, cc, :],
                gt[:].rearrange("p (gl t) c -> p gl c t", t=16),
                axis=mybir.AxisListType.X, op=mybir.AluOpType.max)

        # h_pre = A + S, cast to bf16
        hp = qpool.tile([128, 8, cout], F32, tag="hp")
        nc.vector.tensor_add(hp[:], A[:], S[:])

        # stats over points: PE ones-trick (bf16 inputs, fp32 accumulate)
        ps1 = pstat.tile([1, cout], F32, tag="pstat")
        ps2 = pstat.tile([1, cout], F32, tag="pstat")
        for g in range(8):
            nc.tensor.matmul(ps1[:], ones_col[:], hp[:, g, :],
                             start=(g == 0), stop=(g == 7))
        for g in range(8):
            sg = qpool.tile([128, cout], F32, tag="sqg")
            nc.scalar.square(sg[:], hp[:, g, :])
            nc.tensor.matmul(ps2[:], ones_col[:], sg[:],
                             start=(g == 0), stop=(g == 7))
        return finish_bn(hp, ps1, ps2, cout, li=li)

    def finish_bn(hp, ps1, ps2, cout, li):
        """AllReduce stats, transpose hp, and apply fused BN+lrelu on the
        scalar engine into bf16 ft chunks. li=0 means embedding layer."""
        bname, gname, bename = (f"b{li}", f"g{li}", f"be{li}") if li else ("bse", "ge", "bee")
        stat = mpool.tile([1, 2 * cout], F32, tag="stat")
        nc.scalar.copy(stat[:, 0:cout], ps1[:])
        nc.scalar.copy(stat[:, cout:2 * cout], ps2[:])
        cin_d = dpool.tile([1, 2 * cout], F32, tag="cc_in")
        cout_d = dpool.tile([1, 2 * cout], F32, tag="cc_out")
        nc.gpsimd.dma_start(cin_d[:], stat[:])
        if USE_CC:
            nc.gpsimd.collective_compute(
                "AllReduce", mybir.AluOpType.add,
                replica_groups=[list(range(NCORES))],
                ins=[cin_d.opt()], outs=[cout_d.opt()])
        else:
            nc.gpsimd.dma_start(cout_d[:], cin_d[:])
        statg = mpool.tile([1, 2 * cout], F32, tag="statg")
        nc.gpsimd.dma_start(statg[:], cout_d[:])

        # scale/shift on one partition
        ss = mpool.tile([1, 2 * cout], F32, tag="ss")
        mean = mpool.tile([1, cout], F32, tag="mean")
        var = mpool.tile([1, cout], F32, tag="var")
        nc.scalar.mul(mean[:], statg[:, 0:cout], 1.0 / NPTS)
        nc.scalar.mul(var[:], statg[:, cout:2 * cout], 1.0 / NPTS)   # E[x^2]
        msq = mpool.tile([1, cout], F32, tag="msq")
        nc.scalar.square(msq[:], mean[:])
        nc.vector.scalar_tensor_tensor(var[:], var[:], EPS, msq[:],
                                       op0=mybir.AluOpType.add,
                                       op1=mybir.AluOpType.subtract)
        nc.scalar.activation(msq[:], var[:], mybir.ActivationFunctionType.Sqrt)
        nc.vector.reciprocal(var[:], msq[:])                         # 1/sqrt(var+eps)
        scale_ap, shift_ap = ss[:, 0:cout], ss[:, cout:2 * cout]
        nc.vector.tensor_mul(scale_ap, var[:], vec_sb[gname][:])
        nc.vector.tensor_add(msq[:], mean[:], vec_sb[bname][:])      # mean + b
        nc.vector.tensor_mul(msq[:], msq[:], scale_ap)
        nc.vector.tensor_sub(shift_ap, vec_sb[bename][:], msq[:])

        # transpose scale/shift to per-partition layout [128, nch]
        nch = (cout + 127) // 128
        ssT_s = mpool.tile([128, nch], F32, tag="ssT_s")
        ssT_b = mpool.tile([128, nch], F32, tag="ssT_b")
        for oc in range(nch):
            cw = min(128, cout - oc * 128)
            pss = pT.tile([128, 128], F32, tag="pT")
            nc.tensor.matmul(pss[:cw, 0:1], ss[:, oc * 128:oc * 128 + cw],
                             ones_row[:][:, 0:1])
            nc.scalar.copy(ssT_s[0:cw, oc:oc + 1], pss[:cw, 0:1])
            psb = pT.tile([128, 128], F32, tag="pT")
            nc.tensor.matmul(psb[:cw, 0:1], ss[:, cout + oc * 128:cout + oc * 128 + cw],
                             ones_row[:][:, 0:1])
            nc.scalar.copy(ssT_b[0:cw, oc:oc + 1], psb[:cw, 0:1])

        # transpose hp and apply fused BN + leaky relu on the scalar engine
        fts = []
        for oc in range(nch):
            cw = min(128, cout - oc * 128)
            ft = fpool.tile([cw, N], F32, tag="ft")
            for g in range(8):
                pst = pT.tile([128, 128], F32, tag="pT")
                nc.tensor.transpose(pst[:cw, :], hp[:, g, oc * 128:oc * 128 + cw],
                                    ident[:])
                nc.scalar.activation(ft[:][:, g * 128:(g + 1) * 128], pst[:cw, :],
                                     mybir.ActivationFunctionType.Identity,
                                     bias=ssT_b[0:cw, oc:oc + 1],
                                     scale=ssT_s[0:cw, oc:oc + 1])
            v = ft[:]
            nc.vector.scalar_tensor_tensor(v, v, SLOPE, v,
                                           op0=mybir.AluOpType.mult,
                                           op1=mybir.AluOpType.max)
            fts.append(ft[:])
        return fts

    ft = [XTc[:]]
    ft = emit_layer(ft, 3, 64, 1)
    ft = emit_layer(ft, 64, 128, 2)
    ft = emit_layer(ft, 128, 256, 3)

    # ---------------- embedding + global max pool ----------------
    e = qpool.tile([128, 8, EMB], F32, tag="e", bufs=1)
    for g in range(8):
        pse = pab.tile([128, EMB], F32, tag="psab")
        for kc in range(2):
            nc.tensor.matmul(pse[:], ft[kc][:, g * 128:(g + 1) * 128], we_tiles[kc][:],
                             start=(kc == 0), stop=(kc == 1))
        nc.scalar.copy(e[:, g, :], pse[:])
    ps1 = pstat.tile([1, EMB], F32, tag="pstat")
    ps2 = pstat.tile([1, EMB], F32, tag="pstat")
    for g in range(8):
        nc.tensor.matmul(ps1[:], ones_col[:], e[:, g, :], start=(g == 0), stop=(g == 7))
    for g in range(8):
        sg = qpool.tile([128, EMB], F32, tag="sqg")
        nc.scalar.square(sg[:], e[:, g, :])
        nc.tensor.matmul(ps2[:], ones_col[:], sg[:], start=(g == 0), stop=(g == 7))
    eT = finish_bn(e, ps1, ps2, EMB, li=0)

    # global max pool: reduce each transposed chunk over all 1024 points
    emax = mpool.tile([128, 4], F32, tag="emax")
    for oc in range(4):
        nc.vector.tensor_reduce(emax[:, oc:oc + 1], eT[oc],
                                axis=mybir.AxisListType.X, op=mybir.AluOpType.max)
    psf = pT.tile([4, 128], F32, tag="pT")
    nc.tensor.transpose(psf[:], emax[:], ident[:])
    fin = mpool.tile([4, 128], F32, tag="fin")
    nc.scalar.copy(fin[:], psf[:])
    nc.sync.dma_start(out_t.ap().rearrange("o (j c) -> o j c", j=4), fin[:])


_CACHE = {}


def _build():
    if "nc" in _CACHE:
        return _CACHE["nc"]
    nc = bacc.Bacc("TRN2", target_bir_lowering=False, debug=False,
                   enable_asserts=False, num_devices=NCORES)
    tensors = {"xyz": nc.dram_tensor("xyz", [N, D], F32, kind="ExternalInput"),
               "out": nc.dram_tensor("out", [1, EMB], F32, kind="ExternalOutput")}
    cin = D
    for li, cdim in enumerate(FEATURE_DIMS, start=1):
        tensors[f"W{li}"] = nc.dram_tensor(f"W{li}", [2 * cin, cdim], F32, kind="ExternalInput")
        for pre in ("b", "g", "be"):
            tensors[f"{pre}{li}"] = nc.dram_tensor(f"{pre}{li}", [1, cdim], F32, kind="ExternalInput")
        cin = cdim
    tensors["We"] = nc.dram_tensor("We", [cin, EMB], F32, kind="ExternalInput")
    for nm in ("bse", "ge", "bee"):
        tensors[nm] = nc.dram_tensor(nm, [1, EMB], F32, kind="ExternalInput")

    from contextlib import ExitStack
    with tile.TileContext(nc) as tc:
        for _rep in range(REPEAT):
            with ExitStack() as ctx:
                build_program(nc, tc, tensors, ctx)
    nc.compile()
    _CACHE["nc"] = nc
    return nc


def _in_maps(inputs):
    shared = {}
    for k, v in inputs.items():
        if k == "xyz":
            continue
        a = np.ascontiguousarray(np.asarray(v, dtype=np.float32))
        if a.ndim == 1:
            a = a[None, :]
        shared[k] = a
    xyz = np.asarray(inputs["xyz"], dtype=np.float32)
    maps = []
    for c in range(NCORES):
        m = dict(shared)
        m["xyz"] = np.ascontiguousarray(xyz[c % B])
        maps.append(m)
    return maps


def kernel(**inputs):
    nc = _build()
    res = run_bass_kernel_spmd(nc, _in_maps(inputs), core_ids=list(range(NCORES)))
    return np.stack([res.results[c]["out"][0] for c in range(B)])


# revision 29
# speedup vs baseline: 2.4479x; 1.4821x over previous
"""DGCNN-style EdgeConv point-cloud network on 8 Trainium2 NeuronCores.

Math trick: edge = [center, neigh-center] @ W decomposes as
    h[n,k] = center[n] @ (Wt - Wb) + neigh[n,k] @ Wb        (Wt = W[:C], Wb = W[C:])
so per-layer work collapses to two point-level matmuls (A = F@(Wt-Wb), Bm = F@Wb)
plus a gather of Bm rows by kNN index and a max over the 16 neighbors:
    h_max[n] = A[n] + max_k Bm[idx[n,k]].
Biases fold into the (training-mode) BN shift; BN stats are all-reduced across
the 8 cores (data-parallel over batch; each cloud is processed by two cores,
which leaves the mean/var unchanged).

Sharding: core c processes cloud c % 4 fully. Host gathers outputs of cores 0-3.
"""

import numpy as np

import concourse.bass as bass
import concourse.masks as masks
import concourse.tile as tile
from concourse import bacc, mybir
from concourse.bass_utils import run_bass_kernel_spmd

F32 = mybir.dt.float32
BF16 = mybir.dt.bfloat16
U16 = mybir.dt.uint16
I16 = mybir.dt.int16

B, N, D, KNN = 4, 1024, 3, 16
FEATURE_DIMS = [64, 128, 256]
EMB = 512
NCORES = 8
NEG = -1.0e30
EPS = 1e-5
SLOPE = 0.2
NPTS = NCORES * N          # BN denominator: 8 cores x 1024 pts (each cloud twice)
NT = N // 128              # 8 row tiles of 128 points
GCHUNKS = 8                # gather chunks per layer
IDX_PER_CHUNK = N * KNN // GCHUNKS
DEBUG = False
REPEAT = 1
LRELU_ACT = False
USE_CC = True
USE_GATHER = True


def _canon_out(ap2d, q):
    """Strided out-view placing natural tile-q columns (m = 0..127, point
    n = 128q + m) at canonical positions c(n) = ((n%128)//16)*128 + 16*(n//128)
    + n%16 = (m//16)*128 + 16*q + (m%16): dims [(128,8) m//16, (1,16) m%16]."""
    return bass.AP(tensor=ap2d.tensor, offset=ap2d.offset + 16 * q,
                   ap=[list(ap2d.ap[0]), [128, 8], [1, 16]])


def _sigma_out(ap2d):
    """Strided out-view writing natural column m of tile r to position
    r*128 + sigma^-1(m), sigma^-1(m) = 8*(m%16) + m//16, so that psum row p
    of the distance matmul holds point n = r*128 + sigma(p),
    sigma(p) = 16*(p%8) + p//8."""
    return bass.AP(tensor=ap2d.tensor, offset=ap2d.offset,
                   ap=[list(ap2d.ap[0]), [128, 8], [1, 8], [8, 16]])


def _bcast_free(ap_col, n):
    """View a [P, 1] AP as [P, n] with a step-0 free dim."""
    return bass.AP(tensor=ap_col.tensor, offset=ap_col.offset,
                   ap=[list(ap_col.ap[0]), [0, n]])


def build_program(nc, tc, tensors, ctx):
    xyz = tensors["xyz"]
    out_t = tensors["out"]

    cpool = ctx.enter_context(tc.tile_pool(name="const", bufs=1))
    tpool = ctx.enter_context(tc.tile_pool(name="topk", bufs=2))
    apool = ctx.enter_context(tc.tile_pool(name="acts", bufs=1))
    spool = ctx.enter_context(tc.tile_pool(name="smax", bufs=1))
    bpool = ctx.enter_context(tc.tile_pool(name="bm", bufs=1))
    qpool = ctx.enter_context(tc.tile_pool(name="sq", bufs=2))
    gpool = ctx.enter_context(tc.tile_pool(name="gath", bufs=2))
    fpool = ctx.enter_context(tc.tile_pool(name="ft", bufs=6))
    mpool = ctx.enter_context(tc.tile_pool(name="misc", bufs=1))
    dpool = ctx.enter_context(tc.tile_pool(name="dram", bufs=2, space="DRAM"))
    bmdram = ctx.enter_context(tc.tile_pool(name="bmdram", bufs=1, space="DRAM"))

    pab = ctx.enter_context(tc.tile_pool(name="pab", bufs=2, space="PSUM"))
    pT = ctx.enter_context(tc.tile_pool(name="pT", bufs=1, space="PSUM"))
    pstat = ctx.enter_context(tc.tile_pool(name="pstat", bufs=2, space="PSUM"))

    # ---------------- constants / weights ----------------
    ident = cpool.tile([128, 128], F32)
    masks.make_identity(nc, ident[:])
    ones_col = cpool.tile([128, 1], F32)
    nc.gpsimd.memset(ones_col[:], 1.0)
    ones_row = cpool.tile([1, 128], F32)
    nc.gpsimd.memset(ones_row[:], 1.0)

    # per-layer weight tiles: Wt/Wb K-chunks loaded separately from DRAM
    wsb = {}
    for li, (cin, cout) in enumerate(zip([3] + FEATURE_DIMS[:-1], FEATURE_DIMS), start=1):
        w = tensors[f"W{li}"]
        nch = (cin + 127) // 128
        wt_tiles, wb_tiles = [], []
        for kc in range(nch):
            rows = min(128, cin - kc * 128)
            wt = cpool.tile([rows, cout], F32, tag=f"Wt{li}_{kc}")
            wb = cpool.tile([rows, cout], F32, tag=f"Wb{li}_{kc}")
            nc.sync.dma_start(wt[:], w.ap()[kc * 128:kc * 128 + rows, :])
            nc.sync.dma_start(wb[:], w.ap()[cin + kc * 128:cin + kc * 128 + rows, :])
            wt_tiles.append(wt)
            wb_tiles.append(wb)
        wsb[li] = (wt_tiles, wb_tiles)
    we_tiles = []
    for j in range(2):
        tf = cpool.tile([128, EMB], F32, tag=f"We_{j}")
        nc.sync.dma_start(tf[:], tensors["We"].ap()[j * 128:(j + 1) * 128, :])
        we_tiles.append(tf)

    vec_sb = {}
    for name in ["b1", "g1", "be1", "b2", "g2", "be2", "b3", "g3", "be3",
                 "bse", "ge", "bee"]:
        c = tensors[name].shape[1]
        t = cpool.tile([1, c], F32, tag=f"vec_{name}")
        nc.sync.dma_start(t[:], tensors[name].ap())
        vec_sb[name] = t

    # Wd = Wt - Wb chunks ([C_in_chunk, C_out] each)
    def make_wd(li, cin, cout):
        wt_tiles, wb_tiles = wsb[li]
        chunks = []
        for kc, (wt, wb) in enumerate(zip(wt_tiles, wb_tiles)):
            rows = wt.shape[0]
            wd = cpool.tile([rows, cout], F32, tag=f"wd{li}_{kc}")
            nc.vector.tensor_sub(wd[:], wt[:], wb[:])
            chunks.append((wd[:], wb[:]))  # (Wd, Wb)
        return chunks

    wd_chunks = {1: make_wd(1, 3, 64), 2: make_wd(2, 64, 128), 3: make_wd(3, 128, 256)}

    # ---------------- xyz load + transpose ----------------
    xyz_nat = cpool.tile([128, 8, 3], F32)
    nc.sync.dma_start(xyz_nat[:], xyz.ap().rearrange("(q p) d -> p q d", p=128))
    XT = cpool.tile([3, N], F32)
    XTc = cpool.tile([3, N], F32)
    for q in range(NT):
        ps = pT.tile([3, 128], F32, tag="pT")
        nc.tensor.transpose(ps[:], xyz_nat[:, q, :], ident[:])
        nc.scalar.copy(XT[:][:, q * 128:(q + 1) * 128], ps[:])
        nc.scalar.copy(_canon_out(XTc[:], q), ps[:])

    # squared norms; build matmul operands for negD = 2<xn,xm> - |xm|^2
    xtsq = cpool.tile([3, N], F32)
    nc.scalar.square(xtsq[:], XT[:])
    neg_ones3 = cpool.tile([3, 1], F32)
    nc.gpsimd.memset(neg_ones3[:], -1.0)
    rhs4 = cpool.tile([4, N], F32)
    nc.vector.tensor_copy(rhs4[:][0:3, :], XT[:])
    negsq1 = cpool.tile([1, N], F32)
    for half in range(2):
        psn = pstat.tile([1, 512], F32, tag="pstat")
        nc.tensor.matmul(psn[:], neg_ones3[:], xtsq[:][:, half * 512:(half + 1) * 512])
        nc.scalar.copy(negsq1[:][:, half * 512:(half + 1) * 512], psn[:])
    nc.sync.dma_start(rhs4[:][3:4, :], negsq1[:])
    lhsT4 = cpool.tile([4, N], F32)
    nc.scalar.mul(_sigma_out(lhsT4[:][0:3, :]), XT[:], 2.0)
    ones1 = cpool.tile([1, N], F32)
    nc.gpsimd.memset(ones1[:], 1.0)
    nc.sync.dma_start(lhsT4[:][3:4, :], ones1[:])

    # ---------------- top-16 neighbors ----------------
    idx_all = cpool.tile([128, 128], U16)
    for r in range(NT):
        lhs_ap = lhsT4[:][:, r * 128:(r + 1) * 128]
        negD = tpool.tile([128, N], F32, tag="negD")
        for hh in range(2):
            psD = pab.tile([128, 512], F32, tag="psab")
            nc.tensor.matmul(psD[:], lhs_ap, rhs4[:][:, hh * 512:(hh + 1) * 512])
            nc.scalar.copy(negD[:, hh * 512:(hh + 1) * 512], psD[:])
        vals = tpool.tile([128, 16], F32, tag="vals")
        nc.vector.max(vals[:, 0:8], negD[:])
        nc.vector.max_index(idx_all[:][:, r:r + 57:8], vals[:, 0:8], negD[:])
        negD2 = tpool.tile([128, N], F32, tag="negD2")
        nc.vector.match_replace(negD2[:], vals[:, 0:8], negD[:], NEG)
        nc.vector.max(vals[:, 8:16], negD2[:])
        nc.vector.max_index(idx_all[:][:, 64 + r:64 + r + 57:8], vals[:, 8:16], negD2[:])

    # wrapped index layout for dma_gather: [16 partitions, 1024] u16, replicated x8
    wrapped = cpool.tile([128, N * KNN // 16], U16)
    for k in range(8):
        nc.sync.dma_start(wrapped[:][16 * k:16 * (k + 1), :], idx_all[:])
    if DEBUG:
        d_idx = nc.dram_tensor("dbg_idx", [128, 128], U16, kind="ExternalOutput")
        nc.sync.dma_start(d_idx.ap(), idx_all[:])
        d_wr = nc.dram_tensor("dbg_wrapped", [128, N * KNN // 16], U16, kind="ExternalOutput")
        nc.sync.dma_start(d_wr.ap(), wrapped[:])
        d_xt = nc.dram_tensor("dbg_XT", [3, N], F32, kind="ExternalOutput")
        nc.sync.dma_start(d_xt.ap(), XT[:])
        d_xtc = nc.dram_tensor("dbg_XTc", [3, N], F32, kind="ExternalOutput")
        nc.sync.dma_start(d_xtc.ap(), XTc[:])
        d_l4 = nc.dram_tensor("dbg_lhsT4", [4, N], F32, kind="ExternalOutput")
        nc.sync.dma_start(d_l4.ap(), lhsT4[:])
        d_r4 = nc.dram_tensor("dbg_rhs4", [4, N], F32, kind="ExternalOutput")
        nc.sync.dma_start(d_r4.ap(), rhs4[:])

    # ---------------- generic layer ----------------
    def emit_layer(ft_chunks, cin, cout, li):
        """ft_chunks: list of [K<=128, 1024] bf16 APs (canonical transposed
        features). Returns next-layer ft chunks (bf16)."""
        chunks = wd_chunks[li]
        bmdt = F32  # bf16 gather misbehaves on HW; keep payload fp32

        A = apool.tile([128, 8, cout], F32, tag="A")
        Bm = bpool.tile([128, 8, cout], bmdt, tag="Bm")
        for g in range(8):
            gs = slice(g * 128, (g + 1) * 128)
            psA = pab.tile([128, cout], F32, tag="psab")
            for kc, (wd, _) in enumerate(chunks):
                nc.tensor.matmul(psA[:], ft_chunks[kc][:, gs], wd[:],
                                 start=(kc == 0), stop=(kc == len(chunks) - 1))
            nc.scalar.copy(A[:, g, :], psA[:])
            psB = pab.tile([128, cout], F32, tag="psab")
            for kc, (_, wb) in enumerate(chunks):
                nc.tensor.matmul(psB[:], ft_chunks[kc][:, gs], wb[:],
                                 start=(kc == 0), stop=(kc == len(chunks) - 1))
            nc.scalar.copy(Bm[:, g, :], psB[:])

        # scatter canonical tiles to natural DRAM rows n = 128*(P//16) + 16g + P%16
        bm_d = bmdram.tile([N, cout], bmdt, tag="bmd")
        for g in range(8):
            dst = bass.AP(tensor=bm_d.tensor, offset=bm_d.offset + 16 * g * cout,
                          ap=[[128 * cout, 8], [cout, 16], [1, cout]])
            nc.sync.dma_start(dst, Bm[:, g, :])

        # gather + max over 16 neighbors
        S = spool.tile([128, 8, cout], F32, tag="S")
        wslice = N * KNN // 16 // GCHUNKS
        for cc in range(GCHUNKS):
            gt = gpool.tile([128, IDX_PER_CHUNK // 128, cout], bmdt, tag="gath")
            nc.gpsimd.dma_gather(
                gt[:], bm_d[:],
                wrapped[:][:, cc * wslice:(cc + 1) * wslice].bitcast(I16),
                num_idxs=IDX_PER_CHUNK, num_idxs_reg=IDX_PER_CHUNK,
                elem_size=cout, single_packet=False)
            nc.vector.tensor_reduce(
                S[:, cc, :],
                gt[:].rearrange("p (gl t) c -> p gl c t", t=16),
                axis=mybir.AxisListType.X, op=mybir.AluOpType.max)

        # h_pre = A + S, cast to bf16
        hp = qpool.tile([128, 8, cout], F32, tag="hp")
        nc.vector.tensor_add(hp[:], A[:], S[:])

        # stats over points: PE ones-trick (bf16 inputs, fp32 accumulate)
        ps1 = pstat.tile([1, cout], F32, tag="pstat")
        ps2 = pstat.tile([1, cout], F32, tag="pstat")
        for g in range(8):
            nc.tensor.matmul(ps1[:], ones_col[:], hp[:, g, :],
                             start=(g == 0), stop=(g == 7))
        for g in range(8):
            sg = qpool.tile([128, cout], F32, tag="sqg")
            nc.scalar.square(sg[:], hp[:, g, :])
            nc.tensor.matmul(ps2[:], ones_col[:], sg[:],
                             start=(g == 0), stop=(g == 7))
        return finish_bn(hp, ps1, ps2, cout, li=li)

    def finish_bn(hp, ps1, ps2, cout, li):
        """AllReduce stats, transpose hp, and apply fused BN+lrelu on the
        scalar engine into bf16 ft chunks. li=0 means embedding layer."""
        bname, gname, bename = (f"b{li}", f"g{li}", f"be{li}") if li else ("bse", "ge", "bee")
        stat = mpool.tile([1, 2 * cout], F32, tag="stat")
        nc.scalar.copy(stat[:, 0:cout], ps1[:])
        nc.scalar.copy(stat[:, cout:2 * cout], ps2[:])
        cin_d = dpool.tile([1, 2 * cout], F32, tag="cc_in")
        cout_d = dpool.tile([1, 2 * cout], F32, tag="cc_out")
        nc.sync.dma_start(cin_d[:], stat[:])
        if USE_CC:
            nc.gpsimd.collective_compute(
                "AllReduce", mybir.AluOpType.add,
                replica_groups=[list(range(NCORES))],
                ins=[cin_d.opt()], outs=[cout_d.opt()])
        else:
            nc.sync.dma_start(cout_d[:], cin_d[:])
        statg = mpool.tile([1, 2 * cout], F32, tag="statg")
        nc.sync.dma_start(statg[:], cout_d[:])

        # scale/shift on one partition (mostly DVE to limit cross-engine hops)
        ss = mpool.tile([1, 2 * cout], F32, tag="ss")
        mv = mpool.tile([1, 2 * cout], F32, tag="mv")
        nc.vector.tensor_scalar_mul(mv[:], statg[:], 1.0 / NPTS)     # [mean | E[x^2]]
        mean, ex2 = mv[:, 0:cout], mv[:, cout:2 * cout]
        msq = mpool.tile([1, cout], F32, tag="msq")
        var = mpool.tile([1, cout], F32, tag="var")
        nc.vector.tensor_mul(msq[:], mean, mean)
        nc.vector.scalar_tensor_tensor(var[:], ex2, EPS, msq[:],
                                       op0=mybir.AluOpType.add,
                                       op1=mybir.AluOpType.subtract)
        nc.scalar.activation(msq[:], var[:], mybir.ActivationFunctionType.Sqrt)
        nc.vector.reciprocal(var[:], msq[:])                         # 1/sqrt(var+eps)
        scale_ap, shift_ap = ss[:, 0:cout], ss[:, cout:2 * cout]
        nc.vector.tensor_mul(scale_ap, var[:], vec_sb[gname][:])
        nc.vector.tensor_add(msq[:], mean, vec_sb[bname][:])         # mean + b
        nc.vector.tensor_mul(msq[:], msq[:], scale_ap)
        nc.vector.tensor_sub(shift_ap, vec_sb[bename][:], msq[:])

        # transpose scale/shift to per-partition layout [128, nch]
        nch = (cout + 127) // 128
        ssT_s = mpool.tile([128, nch], F32, tag="ssT_s")
        ssT_b = mpool.tile([128, nch], F32, tag="ssT_b")
        for oc in range(nch):
            cw = min(128, cout - oc * 128)
            pss = pT.tile([128, 128], F32, tag="pT")
            nc.tensor.matmul(pss[:cw, 0:1], ss[:, oc * 128:oc * 128 + cw],
                             ones_row[:][:, 0:1])
            nc.scalar.copy(ssT_s[0:cw, oc:oc + 1], pss[:cw, 0:1])
            psb = pT.tile([128, 128], F32, tag="pT")
            nc.tensor.matmul(psb[:cw, 0:1], ss[:, cout + oc * 128:cout + oc * 128 + cw],
                             ones_row[:][:, 0:1])
            nc.scalar.copy(ssT_b[0:cw, oc:oc + 1], psb[:cw, 0:1])

        # transpose hp and apply fused BN + leaky relu on the scalar engine
        fts = []
        for oc in range(nch):
            cw = min(128, cout - oc * 128)
            ft = fpool.tile([cw, N], F32, tag="ft")
            for g in range(8):
                pst = pT.tile([128, 128], F32, tag="pT")
                nc.tensor.transpose(pst[:cw, :], hp[:, g, oc * 128:oc * 128 + cw],
                                    ident[:])
                nc.scalar.activation(ft[:][:, g * 128:(g + 1) * 128], pst[:cw, :],
                                     mybir.ActivationFunctionType.Identity,
                                     bias=ssT_b[0:cw, oc:oc + 1],
                                     scale=ssT_s[0:cw, oc:oc + 1])
            v = ft[:]
            nc.vector.scalar_tensor_tensor(v, v, SLOPE, v,
                                           op0=mybir.AluOpType.mult,
                                           op1=mybir.AluOpType.max)
            fts.append(ft[:])
        return fts

    ft = [XTc[:]]
    ft = emit_layer(ft, 3, 64, 1)
    ft = emit_layer(ft, 64, 128, 2)
    ft = emit_layer(ft, 128, 256, 3)

    # ---------------- embedding + global max pool ----------------
    e = qpool.tile([128, 8, EMB], F32, tag="e", bufs=1)
    for g in range(8):
        pse = pab.tile([128, EMB], F32, tag="psab")
        for kc in range(2):
            nc.tensor.matmul(pse[:], ft[kc][:, g * 128:(g + 1) * 128], we_tiles[kc][:],
                             start=(kc == 0), stop=(kc == 1))
        nc.scalar.copy(e[:, g, :], pse[:])
    ps1 = pstat.tile([1, EMB], F32, tag="pstat")
    ps2 = pstat.tile([1, EMB], F32, tag="pstat")
    for g in range(8):
        nc.tensor.matmul(ps1[:], ones_col[:], e[:, g, :], start=(g == 0), stop=(g == 7))
    for g in range(8):
        sg = qpool.tile([128, EMB], F32, tag="sqg")
        nc.scalar.square(sg[:], e[:, g, :])
        nc.tensor.matmul(ps2[:], ones_col[:], sg[:], start=(g == 0), stop=(g == 7))
    eT = finish_bn(e, ps1, ps2, EMB, li=0)

    # global max pool: reduce each transposed chunk over all 1024 points
    emax = mpool.tile([128, 4], F32, tag="emax")
    for oc in range(4):
        nc.vector.tensor_reduce(emax[:, oc:oc + 1], eT[oc],
                                axis=mybir.AxisListType.X, op=mybir.AluOpType.max)
    psf = pT.tile([4, 128], F32, tag="pT")
    nc.tensor.transpose(psf[:], emax[:], ident[:])
    fin = mpool.tile([4, 128], F32, tag="fin")
    nc.scalar.copy(fin[:], psf[:])
    nc.sync.dma_start(out_t.ap().rearrange("o (j c) -> o j c", j=4), fin[:])


_CACHE = {}


def _build():
    if "nc" in _CACHE:
        return _CACHE["nc"]
    nc = bacc.Bacc("TRN2", target_bir_lowering=False, debug=False,
                   enable_asserts=False, num_devices=NCORES)
    tensors = {"xyz": nc.dram_tensor("xyz", [N, D], F32, kind="ExternalInput"),
               "out": nc.dram_tensor("out", [1, EMB], F32, kind="ExternalOutput")}
    cin = D
    for li, cdim in enumerate(FEATURE_DIMS, start=1):
        tensors[f"W{li}"] = nc.dram_tensor(f"W{li}", [2 * cin, cdim], F32, kind="ExternalInput")
        for pre in ("b", "g", "be"):
            tensors[f"{pre}{li}"] = nc.dram_tensor(f"{pre}{li}", [1, cdim], F32, kind="ExternalInput")
        cin = cdim
    tensors["We"] = nc.dram_tensor("We", [cin, EMB], F32, kind="ExternalInput")
    for nm in ("bse", "ge", "bee"):
        tensors[nm] = nc.dram_tensor(nm, [1, EMB], F32, kind="ExternalInput")

    from contextlib import ExitStack
    with tile.TileContext(nc) as tc:
        for _rep in range(REPEAT):
            with ExitStack() as ctx:
                build_program(nc, tc, tensors, ctx)
    nc.compile()
    _CACHE["nc"] = nc
    return nc


def _in_maps(inputs):
    shared = {}
    for k, v in inputs.items():
        if k == "xyz":
            continue
        a = np.ascontiguousarray(np.asarray(v, dtype=np.float32))
        if a.ndim == 1:
            a = a[None, :]
        shared[k] = a
    xyz = np.asarray(inputs["xyz"], dtype=np.float32)
    maps = []
    for c in range(NCORES):
        m = dict(shared)
        m["xyz"] = np.ascontiguousarray(xyz[c % B])
        maps.append(m)
    return maps


def kernel(**inputs):
    nc = _build()
    res = run_bass_kernel_spmd(nc, _in_maps(inputs), core_ids=list(range(NCORES)))
    return np.stack([res.results[c]["out"][0] for c in range(B)])
